# revision 1
# baseline (speedup 1.0000x reference)
"""Low-rank attention Trainium2 kernel (8 NeuronCores, SPMD).

Math (reference):
    tmp = relu(x @ W.T + b); U,V,Z,T = split(tmp, 4, axis=1)
    norm = sum(U @ colsum(V)) / n + eps ;  D = 1/norm
    out = concat[(U @ (V.T @ Z)) * D, T]

Sharding: rows of x across 8 cores. Per-core partials (V.T@Z [k,k],
colsum(V), colsum(U)) are AllReduced on-device; each core then computes
its local U @ (VtZ) * D.

Layout trick: x is passed pre-transposed per shard (xT [d, n_loc]) so both
matmul orientations stream straight from HBM with d on partitions.
float32r matmul dtype: full fp32 storage, ~1e-4 matmul rounding, 1 cyc/row.
"""
import sys

sys.path.insert(0, "/opt/trn_rl_repo")
import numpy as np

NCORES = 8
N_ROWS, D_IN, K = 65536, 1024, 256
NLOC = N_ROWS // NCORES      # 8192 rows per core
P = 128
IB = 512                     # i-block width
NB = NLOC // IB              # 16 blocks
EPS = 1e-6
TDEF = 8                     # T-pass blocks deferred to overlap the AllReduce

_built = {}


def _build(d_rows):
    import concourse.bacc as bacc
    import concourse.mybir as mybir
    import concourse.tile as tile

    dt = mybir.dt
    f32, f32r = dt.float32, dt.float32r
    RELU = mybir.ActivationFunctionType.Relu
    DT = d_rows // P
    NSUB = IB // P

    nc = bacc.Bacc("TRN2", target_bir_lowering=False, debug=False, num_devices=NCORES)
    xT = nc.dram_tensor("xT", [d_rows, NLOC], f32r, kind="ExternalInput")
    WT = nc.dram_tensor("WT", [d_rows, 4 * K], f32r, kind="ExternalInput")
    onesc = nc.dram_tensor("onesc", [P, 1], f32r, kind="ExternalInput")
    out = nc.dram_tensor("out", [NLOC, 2 * K], f32, kind="ExternalOutput")

    with tile.TileContext(nc) as tc:
        with (
            tc.tile_pool(name="wp", bufs=1) as wp,
            tc.tile_pool(name="xp", bufs=4) as xp,
            tc.tile_pool(name="up", bufs=1) as up,
            tc.tile_pool(name="vzp", bufs=6) as vzp,
            tc.tile_pool(name="op", bufs=6) as op,
            tc.tile_pool(name="acc", bufs=1) as accp,
            tc.tile_pool(name="ps", bufs=6, space="PSUM") as ps,
            tc.tile_pool(name="dram", bufs=1, space="DRAM") as dram,
        ):
            wt = []
            for kd in range(DT):
                w = wp.tile([P, 4 * K], f32r, tag=f"w{kd}", name=f"w{kd}")
                nc.gpsimd.dma_start(out=w[:], in_=WT[kd * P:(kd + 1) * P, :])
                wt.append(w)
            ones_r = wp.tile([P, 1], f32r, tag="ones_r")
            nc.sync.dma_start(out=ones_r[:], in_=onesc[:, :])
            ones_row = wp.tile([1, P], f32, tag="ones_row")
            nc.vector.memset(ones_row[:], 1.0)

            ut = [up.tile([P, NLOC], f32r, tag=f"ut{h}", name=f"ut{h}") for h in range(2)]
            csu_cols = [accp.tile([P, NB], f32, tag=f"csuc{h}", name=f"csuc{h}") for h in range(2)]
            vtz_acc = [accp.tile([P, K], f32, tag=f"vtza{h}", name=f"vtza{h}") for h in range(2)]
            csv_acc = accp.tile([1, K], f32, tag="csva")

            # ---- phase 1: projection + partial reductions ----
            for ib in range(NB):
                xt = []
                for kd in range(DT):
                    t = xp.tile([P, IB], f32r, tag=f"x{kd}", name=f"x{kd}")
                    nc.sync.dma_start(
                        out=t[:], in_=xT[kd * P:(kd + 1) * P, ib * IB:(ib + 1) * IB]
                    )
                    xt.append(t)
                # U^T [k1, i] — stationary Wu^T, moving x^T; relu on ACT with
                # free-dim running sum (colsum_U partial) via accum_out.
                for h in range(2):
                    pu = ps.tile([P, IB], f32, tag="work")
                    for kd in range(DT):
                        nc.tensor.matmul(
                            pu[:], wt[kd][:, h * P:(h + 1) * P], xt[kd][:],
                            start=(kd == 0), stop=(kd == DT - 1),
                        )
                    nc.scalar.activation(
                        ut[h][:, ib * IB:(ib + 1) * IB], pu[:], RELU,
                        accum_out=csu_cols[h][:, ib:ib + 1],
                    )
                # V|Z and T in natural [i, j] layout per 128-row subtile
                vz_tiles = []
                for s in range(NSUB):
                    i0 = ib * IB + s * P
                    pvz = ps.tile([P, IB], f32, tag="work")
                    for kd in range(DT):
                        nc.tensor.matmul(
                            pvz[:], xt[kd][:, s * P:(s + 1) * P], wt[kd][:, K:3 * K],
                            start=(kd == 0), stop=(kd == DT - 1),
                        )
                    vz = vzp.tile([P, 2 * K], f32r, tag="vz")
                    nc.vector.tensor_relu(vz[:], pvz[:])
                    vz_tiles.append(vz)
                    if ib < NB - TDEF:
                        pt = ps.tile([P, K], f32, tag="work")
                        for kd in range(DT):
                            nc.tensor.matmul(
                                pt[:], xt[kd][:, s * P:(s + 1) * P], wt[kd][:, 3 * K:4 * K],
                                start=(kd == 0), stop=(kd == DT - 1),
                            )
                        ot = op.tile([P, K], f32, tag="ot")
                        nc.vector.tensor_relu(ot[:], pt[:])
                        nc.sync.dma_start(out=out[i0:i0 + P, K:2 * K], in_=ot[:])
                # VtZ partial: contract i (partitions) over this block
                for h in range(2):
                    pz = ps.tile([P, K], f32, tag="work")
                    for s in range(NSUB):
                        nc.tensor.matmul(
                            pz[:], vz_tiles[s][:, h * P:(h + 1) * P],
                            vz_tiles[s][:, K:2 * K],
                            start=(s == 0), stop=(s == NSUB - 1),
                        )
                    if ib == 0:
                        nc.vector.tensor_copy(vtz_acc[h][:], pz[:])
                    else:
                        nc.vector.tensor_add(vtz_acc[h][:], vtz_acc[h][:], pz[:])
                # colsum_V partial via ones-matmul
                pcs = ps.tile([1, K], f32, tag="work")
                for s in range(NSUB):
                    nc.tensor.matmul(
                        pcs[:], ones_r[:], vz_tiles[s][:, 0:K],
                        start=(s == 0), stop=(s == NSUB - 1),
                    )
                if ib == 0:
                    nc.vector.tensor_copy(csv_acc[:], pcs[:])
                else:
                    nc.vector.tensor_add(csv_acc[:], csv_acc[:], pcs[:])

            # ---- phase 2: AllReduce the [k,k]+[k]+[k] partials ----
            csu = [accp.tile([P, 1], f32, tag=f"csu{h}", name=f"csu{h}") for h in range(2)]
            for h in range(2):
                nc.vector.reduce_sum(csu[h][:], csu_cols[h][:], axis=mybir.AxisListType.X)
            bin_ = dram.tile([2 * P + 3, K], f32)
            bout = dram.tile([2 * P + 3, K], f32)
            for h in range(2):
                nc.sync.dma_start(out=bin_[h * P:(h + 1) * P, :], in_=vtz_acc[h][:])
            nc.sync.dma_start(out=bin_[2 * P:2 * P + 1, :], in_=csv_acc[:])
            for h in range(2):
                nc.sync.dma_start(
                    out=bin_[2 * P + 1 + h, 0:P].rearrange("(p one) -> p one", one=1),
                    in_=csu[h][:],
                )
            nc.gpsimd.collective_compute(
                "AllReduce", mybir.AluOpType.add,
                replica_groups=[list(range(NCORES))],
                ins=[bin_.opt()], outs=[bout.opt()],
            )
            # ---- deferred T-pass: keeps PE busy/warm during the AllReduce ----
            for ib in range(NB - TDEF, NB):
                xt = []
                for kd in range(DT):
                    t = xp.tile([P, IB], f32r, tag=f"x{kd}", name=f"xd{kd}")
                    nc.sync.dma_start(
                        out=t[:], in_=xT[kd * P:(kd + 1) * P, ib * IB:(ib + 1) * IB]
                    )
                    xt.append(t)
                for s in range(NSUB):
                    i0 = ib * IB + s * P
                    pt = ps.tile([P, K], f32, tag="work")
                    for kd in range(DT):
                        nc.tensor.matmul(
                            pt[:], xt[kd][:, s * P:(s + 1) * P], wt[kd][:, 3 * K:4 * K],
                            start=(kd == 0), stop=(kd == DT - 1),
                        )
                    ot = op.tile([P, K], f32, tag="ot")
                    nc.vector.tensor_relu(ot[:], pt[:])
                    nc.sync.dma_start(out=out[i0:i0 + P, K:2 * K], in_=ot[:])

            # ---- phase 3: D = 1/(csU.csV/n + eps); scale VtZ ----
            vtzf = [accp.tile([P, K], f32, tag=f"vtzf{h}", name=f"vtzf{h}") for h in range(2)]
            for h in range(2):
                nc.sync.dma_start(out=vtzf[h][:], in_=bout[h * P:(h + 1) * P, :])
            csvt = accp.tile([P, 2], f32, tag="csvt")
            nc.sync.dma_start(out=csvt[:], in_=bout[2 * P, :].rearrange("(t p) -> p t", p=P))
            csut = accp.tile([P, 2], f32, tag="csut")
            nc.sync.dma_start(
                out=csut[:], in_=bout[2 * P + 1:2 * P + 3, 0:P].rearrange("t p -> p t")
            )
            pdot = ps.tile([1, 1], f32, tag="work")
            for h in range(2):
                nc.tensor.matmul(
                    pdot[:], csut[:, h:h + 1], csvt[:, h:h + 1],
                    start=(h == 0), stop=(h == 1),
                )
            dsb = accp.tile([1, 1], f32, tag="dsb")
            nc.vector.tensor_scalar(
                out=dsb[:], in0=pdot[:], scalar1=1.0 / N_ROWS, scalar2=EPS,
                op0=mybir.AluOpType.mult, op1=mybir.AluOpType.add,
            )
            nc.vector.reciprocal(dsb[:], dsb[:])
            pb = ps.tile([P, 1], f32, tag="work")
            nc.tensor.matmul(pb[:], ones_row[:], dsb[:], start=True, stop=True)
            dbc = accp.tile([P, 1], f32, tag="dbc")
            nc.vector.tensor_copy(dbc[:], pb[:])
            vtzr = [accp.tile([P, K], f32r, tag=f"vtzr{h}", name=f"vtzr{h}") for h in range(2)]
            for h in range(2):
                nc.vector.tensor_scalar_mul(vtzr[h][:], vtzf[h][:], dbc[:])

            # ---- phase 4: res = U @ (VtZ * D), written row-natural ----
            for ib in range(NB):
                for s in range(NSUB):
                    i0 = ib * IB + s * P
                    pr = ps.tile([P, K], f32, tag="work")
                    for h in range(2):
                        nc.tensor.matmul(
                            pr[:], ut[h][:, i0:i0 + P], vtzr[h][:],
                            start=(h == 0), stop=(h == 1),
                        )
                    orow = op.tile([P, K], f32, tag="ot")
                    nc.vector.tensor_copy(orow[:], pr[:])
                    nc.sync.dma_start(out=out[i0:i0 + P, 0:K], in_=orow[:])

    nc.compile()
    return nc


def _get_nc(d_rows):
    if d_rows not in _built:
        _built[d_rows] = _build(d_rows)
    return _built[d_rows]


def _run(x, W, b, trace=False, trace_cores=None):
    from concourse.bass_utils import run_bass_kernel_spmd

    x = np.ascontiguousarray(x, dtype=np.float32)
    W = np.ascontiguousarray(W, dtype=np.float32)
    b = np.asarray(b, dtype=np.float32)
    if np.any(b):
        d_rows = 1152  # pad contraction: extra ones-row in x picks up b from W
        WT_full = np.zeros((d_rows, 4 * K), np.float32)
        WT_full[:D_IN] = W.T
        WT_full[D_IN] = b
    else:
        d_rows = D_IN
        WT_full = np.ascontiguousarray(W.T)
    nc = _get_nc(d_rows)
    in_maps = []
    for c in range(NCORES):
        xs = x[c * NLOC:(c + 1) * NLOC]
        if d_rows == D_IN:
            xTs = np.ascontiguousarray(xs.T)
        else:
            xTs = np.zeros((d_rows, NLOC), np.float32)
            xTs[:D_IN] = xs.T
            xTs[D_IN] = 1.0
        in_maps.append({"xT": xTs, "WT": WT_full, "onesc": np.ones((P, 1), np.float32)})
    res = run_bass_kernel_spmd(
        nc, in_maps, list(range(NCORES)),
        trace=trace, **({"trace_cores": trace_cores} if trace_cores else {}),
    )
    full = np.concatenate([res.results[c]["out"] for c in range(NCORES)], axis=0)
    return full, res


def kernel(x, W, b):
    full, _ = _run(x, W, b)
    return full



# revision 3
# speedup vs baseline: 1.3423x; 1.3423x over previous
"""Low-rank attention Trainium2 kernel (8 NeuronCores, SPMD), bf16 edition.

Math (reference):
    tmp = relu(x @ W.T + b); U,V,Z,T = split(tmp, 4, axis=1)
    norm = sum(U @ colsum(V)) / n + eps ;  D = 1/norm
    out = concat[(U @ (V.T @ Z)) * D, T]

Sharding: rows of x across 8 cores. Per-core partials (V.T@[Z|1] [k,k+1]
which embeds colsum(V) in its last column, plus colsum(U)) are AllReduced
on-device; each core then computes its local U @ (VtZ * D).

Layout: everything except V/Z is computed TRANSPOSED ([feature, row]) so the
weight block is the stationary matmul operand and is reused across 4 moving
512-wide row chunks. V/Z need rows on partitions for the V^T@Z contraction,
so they alone use the natural layout (x-slice stationary). The kernel emits
outT [2k, n_loc]; the host transposes back. bf16 operands halve DMA and get
FWL weight loads; fp32 PSUM accumulation keeps rel-err ~5e-3.

All of x stays resident in SBUF (16 MB bf16), so the whole T-pass defers
until after the AllReduce launch with no HBM re-read, hiding the collective
behind ~55us of PE work.
"""
import sys

sys.path.insert(0, "/opt/trn_rl_repo")
import ml_dtypes
import numpy as np

NCORES = 8
N_ROWS, D_IN, K = 65536, 1024, 256
NLOC = N_ROWS // NCORES      # 8192 rows per core
P = 128
IG = 2048                    # i-group (rows handled per outer iteration)
NG = NLOC // IG              # 4 groups
IC = 512                     # i-chunk = one PSUM bank of fp32
NIC = IG // IC               # 4 chunks per group
NS = IG // P                 # 16 row-subtiles per group
EPS = 1e-6

_built = {}


def _build(d_rows, resident):
    import concourse.bacc as bacc
    import concourse.mybir as mybir
    import concourse.tile as tile

    dt = mybir.dt
    f32, bf16 = dt.float32, dt.bfloat16
    RELU = mybir.ActivationFunctionType.Relu
    DT = d_rows // P

    nc = bacc.Bacc("TRN2", target_bir_lowering=False, debug=False, num_devices=NCORES)
    xT = nc.dram_tensor("xT", [d_rows, NLOC], bf16, kind="ExternalInput")
    WT = nc.dram_tensor("WT", [d_rows, 4 * K], bf16, kind="ExternalInput")
    outT = nc.dram_tensor("outT", [2 * K, NLOC], f32, kind="ExternalOutput")

    with tile.TileContext(nc) as tc:
        with (
            tc.tile_pool(name="wp", bufs=1) as wp,
            tc.tile_pool(name="xp", bufs=1) as xp,
            tc.tile_pool(name="up", bufs=1) as up,
            tc.tile_pool(name="vzp", bufs=4) as vzp,
            tc.tile_pool(name="op", bufs=6) as op,
            tc.tile_pool(name="acc", bufs=1) as accp,
            tc.tile_pool(name="ps", bufs=6, space="PSUM") as ps,
            tc.tile_pool(name="psv", bufs=2, space="PSUM") as psv,
            tc.tile_pool(name="dram", bufs=1, space="DRAM") as dram,
        ):
            wt = []
            for kd in range(DT):
                w = wp.tile([P, 4 * K], bf16, tag=f"w{kd}", name=f"w{kd}")
                nc.gpsimd.dma_start(out=w[:], in_=WT[kd * P:(kd + 1) * P, :])
                wt.append(w)
            ones_row = wp.tile([1, P], f32, tag="ones_row")
            nc.vector.memset(ones_row[:], 1.0)

            ut = [up.tile([P, NLOC], bf16, tag=f"ut{h}", name=f"ut{h}") for h in range(2)]
            csu_cols = [
                accp.tile([P, NG * NIC], f32, tag=f"csuc{h}", name=f"csuc{h}")
                for h in range(2)
            ]
            vtz_acc = [
                accp.tile([P, K + 1], f32, tag=f"vtza{h}", name=f"vtza{h}")
                for h in range(2)
            ]

            # x tiles: resident path holds the whole shard in SBUF.
            def new_xtiles(g):
                xt = []
                for kd in range(DT):
                    t = xp.tile(
                        [P, IG], bf16,
                        tag=(f"x{g}_{kd}" if resident else f"x{kd}"),
                        bufs=(1 if resident else 2),
                        name=f"x{g}_{kd}",
                    )
                    nc.sync.dma_start(
                        out=t[:], in_=xT[kd * P:(kd + 1) * P, g * IG:(g + 1) * IG]
                    )
                    xt.append(t)
                return xt

            xg = [new_xtiles(g) for g in range(NG)] if resident else [None] * NG

            # transposed-layout projection for one 128-wide feature block jb:
            # psum[jb, ic] += wt[kd][:,jb].T @ xt[kd][:,ic]  (weight stationary,
            # reused across the NIC moving chunks)
            def tpass(g, jb, xt):
                pts = [ps.tile([P, IC], f32, tag="work", name=f"pt{i}") for i in range(NIC)]
                for kd in range(DT):
                    for ic in range(NIC):
                        nc.tensor.matmul(
                            pts[ic][:], wt[kd][:, jb * P:(jb + 1) * P],
                            xt[kd][:, ic * IC:(ic + 1) * IC],
                            start=(kd == 0), stop=(kd == DT - 1),
                            skip_group_check=True,
                        )
                for ic in range(NIC):
                    i0 = g * IG + ic * IC
                    if jb < 2:  # U features, keep transposed in SBUF + colsum(U)
                        nc.scalar.activation(
                            ut[jb][:, i0:i0 + IC], pts[ic][:], RELU,
                            accum_out=csu_cols[jb][:, g * NIC + ic:g * NIC + ic + 1],
                        )
                    else:       # T features, straight to output rows 256:512
                        ot = op.tile([P, IC], f32, tag="ot", name="ot")
                        nc.vector.tensor_relu(ot[:], pts[ic][:])
                        nc.sync.dma_start(
                            out=outT[K + (jb - 6) * P:K + (jb - 5) * P, i0:i0 + IC],
                            in_=ot[:],
                        )

            # ---- phase 1: projection + VtZ/colsum partials ----
            for g in range(NG):
                xt = xg[g] if resident else new_xtiles(g)
                for jb in ([0, 1] if resident else [0, 1, 6, 7]):
                    tpass(g, jb, xt)
                # natural-layout V|Z for this group's 16 row-subtiles, with a
                # ones column so V^T@[Z|1] also yields colsum(V) in column K.
                pvtz = [
                    psv.tile([P, K + 1], f32, tag="vtz", name=f"pvtz{h}")
                    for h in range(2)
                ]
                for s in range(NS):
                    pvz = ps.tile([P, 2 * K], f32, tag="work", name="pvz")
                    for kd in range(DT):
                        nc.tensor.matmul(
                            pvz[:], xt[kd][:, s * P:(s + 1) * P], wt[kd][:, K:3 * K],
                            start=(kd == 0), stop=(kd == DT - 1),
                        )
                    vz = vzp.tile([P, 2 * K + 1], bf16, tag="vz", name="vz")
                    nc.vector.tensor_relu(vz[:, 0:2 * K], pvz[:])
                    nc.vector.memset(vz[:, 2 * K:2 * K + 1], 1.0)
                    for h in range(2):
                        nc.tensor.matmul(
                            pvtz[h][:], vz[:, h * P:(h + 1) * P], vz[:, K:2 * K + 1],
                            start=(s == 0), stop=(s == NS - 1),
                            skip_group_check=True,
                        )
                for h in range(2):
                    if g == 0:
                        nc.vector.tensor_copy(vtz_acc[h][:], pvtz[h][:])
                    else:
                        nc.vector.tensor_add(vtz_acc[h][:], vtz_acc[h][:], pvtz[h][:])

            # ---- phase 2: AllReduce [2P+2, K+1] of VtZ|csV rows + csU rows ----
            csu = [accp.tile([P, 1], f32, tag=f"csu{h}", name=f"csu{h}") for h in range(2)]
            for h in range(2):
                nc.vector.reduce_sum(csu[h][:], csu_cols[h][:], axis=mybir.AxisListType.X)
            bin_ = dram.tile([2 * P + 2, K + 1], f32, name="bin")
            bout = dram.tile([2 * P + 2, K + 1], f32, name="bout")
            for h in range(2):
                nc.sync.dma_start(out=bin_[h * P:(h + 1) * P, :], in_=vtz_acc[h][:])
                nc.sync.dma_start(
                    out=bin_[2 * P + h, 0:P].rearrange("(p one) -> p one", one=1),
                    in_=csu[h][:],
                )
            nc.gpsimd.collective_compute(
                "AllReduce", mybir.AluOpType.add,
                replica_groups=[list(range(NCORES))],
                ins=[bin_.opt()], outs=[bout.opt()],
            )

            # ---- deferred T-pass: keeps PE busy through the AllReduce ----
            if resident:
                for g in range(NG):
                    for jb in (6, 7):
                        tpass(g, jb, xg[g])

            # ---- phase 3: D = 1/(csU.csV/n + eps); vtzr = VtZ * D ----
            vtzf = [
                accp.tile([P, K + 1], f32, tag=f"vtzf{h}", name=f"vtzf{h}")
                for h in range(2)
            ]
            for h in range(2):
                nc.sync.dma_start(out=vtzf[h][:], in_=bout[h * P:(h + 1) * P, :])
            csut = accp.tile([P, 2], f32, tag="csut")
            nc.sync.dma_start(
                out=csut[:], in_=bout[2 * P:2 * P + 2, 0:P].rearrange("t p -> p t")
            )
            pdot = ps.tile([1, 1], f32, tag="work", name="pdot")
            for h in range(2):
                nc.tensor.matmul(
                    pdot[:], csut[:, h:h + 1], vtzf[h][:, K:K + 1],
                    start=(h == 0), stop=(h == 1),
                )
            dsb = accp.tile([1, 1], f32, tag="dsb")
            nc.vector.tensor_scalar(
                out=dsb[:], in0=pdot[:], scalar1=1.0 / N_ROWS, scalar2=EPS,
                op0=mybir.AluOpType.mult, op1=mybir.AluOpType.add,
            )
            nc.vector.reciprocal(dsb[:], dsb[:])
            pb = ps.tile([P, 1], f32, tag="work", name="pb")
            nc.tensor.matmul(pb[:], ones_row[:], dsb[:], start=True, stop=True)
            dbc = accp.tile([P, 1], f32, tag="dbc")
            nc.vector.tensor_copy(dbc[:], pb[:])
            vtzr = [
                accp.tile([P, K], bf16, tag=f"vtzr{h}", name=f"vtzr{h}")
                for h in range(2)
            ]
            for h in range(2):
                nc.vector.tensor_scalar_mul(vtzr[h][:], vtzf[h][:, 0:K], dbc[:])

            # ---- phase 4: resT = (VtZ*D).T @ UT, written transposed ----
            for q in range(2):
                for icg in range(NIC):
                    prs = [
                        ps.tile([P, IC], f32, tag="work", name=f"pr{j}")
                        for j in range(4)
                    ]
                    for h in range(2):
                        for j in range(4):
                            ic = icg * 4 + j
                            nc.tensor.matmul(
                                prs[j][:], vtzr[h][:, q * P:(q + 1) * P],
                                ut[h][:, ic * IC:(ic + 1) * IC],
                                start=(h == 0), stop=(h == 1),
                                skip_group_check=True,
                            )
                    for j in range(4):
                        ic = icg * 4 + j
                        orow = op.tile([P, IC], f32, tag="ot", name="orow")
                        nc.scalar.copy(orow[:], prs[j][:])
                        nc.sync.dma_start(
                            out=outT[q * P:(q + 1) * P, ic * IC:(ic + 1) * IC],
                            in_=orow[:],
                        )

    nc.compile()
    return nc


def _get_nc(d_rows, resident):
    key = (d_rows, resident)
    if key not in _built:
        _built[key] = _build(d_rows, resident)
    return _built[key]


def _run(x, W, b, trace=False, trace_cores=None):
    from concourse.bass_utils import run_bass_kernel_spmd

    bf16 = ml_dtypes.bfloat16
    x = np.ascontiguousarray(x, dtype=np.float32)
    W = np.ascontiguousarray(W, dtype=np.float32)
    b = np.asarray(b, dtype=np.float32)
    if np.any(b):
        # pad contraction: a ones-row in x picks up b from an extra W row
        d_rows, resident = 1152, False
        WT_full = np.zeros((d_rows, 4 * K), bf16)
        WT_full[:D_IN] = W.T.astype(bf16)
        WT_full[D_IN] = b.astype(bf16)
    else:
        d_rows, resident = D_IN, True
        WT_full = np.ascontiguousarray(W.T).astype(bf16)
    nc = _get_nc(d_rows, resident)
    in_maps = []
    for c in range(NCORES):
        xs = x[c * NLOC:(c + 1) * NLOC]
        if resident:
            xTs = np.ascontiguousarray(xs.T).astype(bf16)
        else:
            xTs = np.zeros((d_rows, NLOC), bf16)
            xTs[:D_IN] = xs.T.astype(bf16)
            xTs[D_IN] = 1.0
        in_maps.append({"xT": xTs, "WT": WT_full})
    res = run_bass_kernel_spmd(
        nc, in_maps, list(range(NCORES)),
        trace=trace, **({"trace_cores": trace_cores} if trace_cores else {}),
    )
    full = np.concatenate(
        [np.ascontiguousarray(res.results[c]["outT"].T) for c in range(NCORES)],
        axis=0,
    )
    return full, res


def kernel(x, W, b):
    full, _ = _run(x, W, b)
    return full
